# revision 36
# baseline (speedup 1.0000x reference)
"""HeadFusionAttention Trainium2 kernel (8 NeuronCores, data-parallel over B).

Reference computation (per batch b):
    head_x = 0
    for i in 0..3:                                  # sequential group chain
        cur   = x[:, 256*i:256*(i+1)] + head_x      # [N, 256]
        qkv   = cur @ qkv_w[i].T                    # [N, 768] -> q,k,v [N,256]
        S     = (q @ k.T) * SCALE                   # [N, N]
        P     = softmax(S, axis=-1)
        head_x = P @ v                              # [N, 256]
        y[:, 256*i:256*(i+1)] = head_x
    out = y @ proj_w.T + proj_b                     # [N, 1024]

Sharding: B=16 batches split 2 per core across 8 cores; weights replicated;
zero collectives. All activations are kept transposed on-chip ([feature, n]
layout) so every matmul consumes operands in natural [K, M]/[K, N] form and
no on-chip transposes are needed:
  - qkv^T = wT.T-matmul(cur^T)            (q^T, k^T in [d, n], fp32r)
  - v is produced directly in [n, d] via swapping matmul operands (bf16)
  - S^T   = k^T-matmul(q^T)  ([k_n, q_n]); softmax runs along partitions:
    exp via ACT (scale folded in, no max subtraction -- scores are O(1))
    writing bf16 pt; denominators via an fp32 tree-sum of the pt tiles
    plus ONE ones-vector matmul per query half (instead of 8 accumulating
    matmuls), reciprocal broadcast across partitions on the idle GpSimd
    engine (partition_broadcast) instead of an all-ones matmul
  - head_x^T = v-matmul(P^T) in bf16 operands (f32 psum accumulate),
    normalized by the broadcast reciprocal denominators
    (reciprocal_approx_fast: denominators are sums of exps, no edge cases)
  - projection runs entirely in bf16 (pw, head_x); the two groups of a
    pair share one PSUM accumulation, so out_acc sees one copy + one add
  - out^T accumulates in bf16 (bias folded into the first partial);
    stored as bf16, host casts + transposes back.
Scheduling: engine queues execute in emission order, so the two batches of
the per-core shard are emitted interleaved per group (fills the PE during
either batch's sequential group-boundary stalls), and each projection pair
is emitted after BOTH batches' attention so one batch's pair matmuls cover
the other's softmax tail (recip -> partition_broadcast -> normalize).
PSUM->SBUF moves and elementwise ops are emitted on the 'any' engine so the
tile scheduler balances them across DVE/Act/Pool instead of serializing on
the vector engine. Simulated device time 243 us/core, PE occupancy 88%
(205 us intrinsic PE floor for the 16.1 GFLOP/core at fp32r full rate).
"""

import numpy as np

B, N, DIM = 16, 1024, 1024
G = 4
G_DIM = 256
SCALE = 128 ** -0.5
N_CORES = 8
B_PER = B // N_CORES  # 2

P = 128          # SBUF partitions
FH = 512         # free-dim half (psum bank: 512 fp32)
USE_F32R = True  # fp32r matmuls: full-rate single pass (vs 4x slower fp32)


def build_nc(use_f32r=USE_F32R):
    from contextlib import ExitStack

    import concourse.mybir as mybir
    import concourse.tile as tile
    from concourse import bacc

    f32 = mybir.dt.float32
    bf16 = mybir.dt.bfloat16
    # float32r: same 4-byte layout as fp32 but single-pass full-rate matmul.
    # The BIR verifier requires every matmul operand's producer to emit
    # float32r, so all matmul-feeding tiles/params are typed float32r.
    mdt = mybir.dt.float32r if use_f32r else f32

    # Bacc (vs plain Bass) runs the wait-splitting passes walrus requires
    nc = bacc.Bacc()

    xT = nc.declare_dram_parameter("xT", [B_PER, DIM, N], mdt, isOutput=False)
    wqkvT = nc.declare_dram_parameter("wqkvT", [G, G_DIM, 3 * G_DIM], mdt, isOutput=False)
    pwTh = nc.declare_dram_parameter("pwT", [DIM, DIM], bf16, isOutput=False)
    pb = nc.declare_dram_parameter("pb", [P, DIM // P], f32, isOutput=False)
    outT = nc.declare_dram_parameter("outT", [B_PER, DIM, N], bf16, isOutput=True)

    Exp = mybir.ActivationFunctionType.Exp
    Ident = mybir.ActivationFunctionType.Identity

    with tile.TileContext(nc) as tc, ExitStack() as ctx:
        consts = ctx.enter_context(tc.tile_pool(name="consts", bufs=1))
        pw_pool = ctx.enter_context(tc.tile_pool(name="pw_pool", bufs=2))
        acc_pool = ctx.enter_context(tc.tile_pool(name="acc_pool", bufs=2))
        cur_pool = ctx.enter_context(tc.tile_pool(name="cur_pool", bufs=3))
        qk_pool = ctx.enter_context(tc.tile_pool(name="qk_pool", bufs=2))
        v_pool = ctx.enter_context(tc.tile_pool(name="v_pool", bufs=2))
        pt_pool = ctx.enter_context(tc.tile_pool(name="pt_pool", bufs=4))
        pts_pool = ctx.enter_context(tc.tile_pool(name="pts_pool", bufs=4))
        hx_pool = ctx.enter_context(tc.tile_pool(name="hx_pool", bufs=4))
        sm_pool = ctx.enter_context(tc.tile_pool(name="sm_pool", bufs=2))

        ps_mm = ctx.enter_context(tc.tile_pool(name="ps_mm", bufs=2, space="PSUM"))
        ps_s = ctx.enter_context(tc.tile_pool(name="ps_s", bufs=3, space="PSUM"))
        ps_pv = ctx.enter_context(tc.tile_pool(name="ps_pv", bufs=2, space="PSUM"))
        ps_den = ctx.enter_context(tc.tile_pool(name="ps_den", bufs=1, space="PSUM"))

        # ---- constants ----
        # qkv weights, transposed: [d partition, group, d-subtile, e].
        # Per-group DMAs, group 0 first, so the first matmuls start sooner.
        wq_sb = consts.tile([P, G, 2, 3 * G_DIM], mdt)
        # Startup ordering: the first qkv chain needs only the first q/k
        # e-chunk of group-0 weights and the first half of batch-0's input
        # -- land exactly those two before everything else.
        w0 = wqkvT[0].rearrange("(ds p) e -> p ds e", p=P)
        nc.sync.dma_start(out=wq_sb[:, 0, :, 0:P], in_=w0[:, :, 0:P])
        cur0 = cur_pool.tile([P, 2, N], mdt, tag="cur")
        x0 = xT[0, 0:G_DIM].rearrange("(ds p) n -> p ds n", p=P)
        nc.sync.dma_start(out=cur0[:, :, 0:FH], in_=x0[:, :, 0:FH])
        nc.sync.dma_start(out=wq_sb[:, 0, :, P : 3 * G_DIM], in_=w0[:, :, P : 3 * G_DIM])
        nc.sync.dma_start(out=cur0[:, :, FH:N], in_=x0[:, :, FH:N])
        for g in range(1, G):
            nc.sync.dma_start(
                out=wq_sb[:, g],
                in_=wqkvT[g].rearrange("(ds p) e -> p ds e", p=P),
            )
        pb_sb = consts.tile([P, DIM // P], f32)
        nc.sync.dma_start(out=pb_sb, in_=pb[:, :])
        # memset can't write float32r directly; stage via f32 + copy
        ones_f32 = consts.tile([P, 1], f32)
        nc.vector.memset(ones_f32, 1.0)
        ones_col = consts.tile([P, 1], mdt)
        nc.vector.tensor_copy(ones_col, ones_f32)

        # Per-batch state; the two batches are emitted INTERLEAVED per group:
        # batch 1's matmuls are queued (in-order engine queues) right behind
        # batch 0's attention tail, so the PE never drains at the sequential
        # group boundaries of either batch.
        out_accs, curs, pairs = [], [], []
        for b in range(B_PER):
            out_acc = acc_pool.tile([P, DIM // P, N], bf16, name=f"out_acc{b}")
            out_accs.append(out_acc)
            pairs.append([])
            if b == 0:
                curs.append(cur0)
            else:
                cur = cur_pool.tile([P, 2, N], mdt, tag="cur", name=f"cur_b{b}")
                nc.sync.dma_start(
                    out=cur, in_=xT[b, 0:G_DIM].rearrange("(ds p) n -> p ds n", p=P)
                )
                curs.append(cur)

        for i in range(G):
            # projection weight slice for this group (shared by both batches)
            pw_s = pw_pool.tile([P, 2, DIM], bf16)
            nc.sync.dma_start(
                out=pw_s,
                in_=pwTh[G_DIM * i : G_DIM * (i + 1)].rearrange(
                    "(ds p) e -> p ds e", p=P
                ),
            )
            for b in range(B_PER):
                out_acc = out_accs[b]
                cur = curs[b]

                # ---- A: q^T, k^T [e-chunk, n] = w_qk.T-matmul(cur^T);
                # h-outer so the first chains only need the first half of
                # cur (the group-0 input DMA is split per half) ----
                qkT = qk_pool.tile([P, 4, N], mdt)
                v_sb = v_pool.tile([P, 8, G_DIM], bf16)
                for h in range(2):
                    for ec in range(4):
                        ps = ps_mm.tile([P, FH], f32, tag="ps_mm")
                        for ds in range(2):
                            nc.tensor.matmul(
                                ps,
                                (wq_sb[:, i, ds, P * ec : P * (ec + 1)]),
                                (cur[:, ds, FH * h : FH * (h + 1)]),
                                start=(ds == 0),
                                stop=(ds == 1),
                            )
                        nc.any.tensor_copy(qkT[:, ec, FH * h : FH * (h + 1)], ps)

                    # ---- B: v [n-chunk, d] = cur-matmul(w_v), bf16 ----
                    for nk in range(4 * h, 4 * h + 4):
                        ps = ps_mm.tile([P, FH], f32, tag="ps_mm")
                        for ds in range(2):
                            nc.tensor.matmul(
                                ps[:, :G_DIM],
                                (cur[:, ds, P * nk : P * (nk + 1)]),
                                (wq_sb[:, i, ds, 2 * G_DIM : 3 * G_DIM]),
                                start=(ds == 0),
                                stop=(ds == 1),
                            )
                        nc.any.tensor_copy(v_sb[:, nk], ps[:, :G_DIM])

                # next group's x slice (overwritten into cur_next, then += hx)
                cur_next = None
                if i + 1 < G:
                    cur_next = cur_pool.tile([P, 2, N], mdt, tag="cur")
                    nc.sync.dma_start(
                        out=cur_next,
                        in_=xT[b, G_DIM * (i + 1) : G_DIM * (i + 2)].rearrange(
                            "(ds p) n -> p ds n", p=P
                        ),
                    )

                hx = hx_pool.tile([P, 2, N], bf16)

                # ---- attention, one q-half at a time ----
                for h in range(2):
                    pv_ps = [
                        ps_pv.tile([P, FH], f32, tag="ps_pv", name=f"pv_ps{dc}")
                        for dc in range(2)
                    ]
                    den_ps = ps_den.tile([1, FH], f32, tag="ps_den")
                    # fp32r partial sums of pt tiles (tree) for the denominator
                    t_sum = [None] * 4
                    for kc in range(8):
                        s_ps = ps_s.tile([P, FH], f32, tag="ps_s")
                        for ds in range(2):
                            nc.tensor.matmul(
                                s_ps,
                                (qkT[:, 2 + ds, P * kc : P * (kc + 1)]),
                                (qkT[:, ds, FH * h : FH * (h + 1)]),
                                start=(ds == 0),
                                stop=(ds == 1),
                            )
                        pt = pt_pool.tile([P, FH], bf16)
                        nc.scalar.activation(pt, s_ps, Exp, scale=SCALE)
                        for dc in range(2):
                            nc.tensor.matmul(
                                pv_ps[dc],
                                (v_sb[:, kc, P * dc : P * (dc + 1)]),
                                (pt),
                                start=(kc == 0),
                                stop=(kc == 7),
                            )
                        if kc % 2 == 0:
                            prev_pt = pt
                        else:
                            t = pts_pool.tile([P, FH], mdt, tag="t_sum")
                            nc.any.tensor_add(t, prev_pt, pt)
                            t_sum[kc // 2] = t
                    nc.any.tensor_add(t_sum[0], t_sum[0], t_sum[1])
                    nc.any.tensor_add(t_sum[2], t_sum[2], t_sum[3])
                    nc.any.tensor_add(t_sum[0], t_sum[0], t_sum[2])
                    nc.tensor.matmul(
                        den_ps, (ones_col), (t_sum[0]), start=True, stop=True
                    )

                    # reciprocal on row 0, broadcast across partitions on the
                    # (otherwise idle) GpSimd engine
                    rec_row = sm_pool.tile([1, FH], f32, tag="rec_row")
                    nc.vector.reciprocal_approx_fast(rec_row, den_ps)
                    rec_b = sm_pool.tile([P, FH], f32, tag="rec_b")
                    nc.gpsimd.partition_broadcast(rec_b, rec_row)

                    for dc in range(2):
                        nc.vector.tensor_mul(
                            hx[:, dc, FH * h : FH * (h + 1)], pv_ps[dc], rec_b
                        )
                        if cur_next is not None:
                            nc.vector.tensor_add(
                                cur_next[:, dc, FH * h : FH * (h + 1)],
                                cur_next[:, dc, FH * h : FH * (h + 1)],
                                hx[:, dc, FH * h : FH * (h + 1)],
                            )

                pairs[b].append((hx, i, pw_s))
                curs[b] = cur_next

            # ---- paired projection partials: groups (0,1) after group 1,
            # groups (2,3) after group 3 (with the final out^T stores) --
            # emitted after BOTH batches' attention so batch 0's pair
            # matmuls fill the PE while batch 1's softmax tail (recip ->
            # broadcast -> normalize) drains, and vice versa. Each pair
            # shares one PSUM accumulation, so out_acc sees one copy + one
            # add instead of four adds. ----
            if i in (1, G - 1):
                for b in range(B_PER):
                    _emit_proj_pair(
                        nc, ps_mm, out_accs[b], pairs[b], pb_sb, f32, Ident,
                        outT=outT if i == G - 1 else None, b=b,
                    )
                    pairs[b] = []

    nc.finalize()
    return nc


def _emit_proj_pair(nc, ps_mm, out_acc, pair, pb_sb, f32, Ident,
                    outT=None, b=0):
    """out_acc[:, ec, :] (+)= sum over (hx, gi, pw) in pair of pw.T-mm(hx).
    The pair shares one PSUM accumulation chain per (ec, h). The first pair
    (containing group 0) lands via activation+bias; the second accumulates
    with one add. When outT is given, DMA each ec chunk right after its
    final accumulation so the store overlaps remaining compute."""
    first = pair[0][1] == 0
    DIM_ = out_acc.shape[1] * 128
    nmm = 2 * len(pair)
    for ec in range(DIM_ // 128):
        for h in range(2):
            ps = ps_mm.tile([128, FH], f32, tag="ps_mm")
            k = 0
            for hx, gi, pw_s in pair:
                for ds in range(2):
                    nc.tensor.matmul(
                        ps,
                        (pw_s[:, ds, 128 * ec : 128 * (ec + 1)]),
                        (hx[:, ds, FH * h : FH * (h + 1)]),
                        start=(k == 0),
                        stop=(k == nmm - 1),
                    )
                    k += 1
            dst = out_acc[:, ec, FH * h : FH * (h + 1)]
            if first:
                nc.scalar.activation(dst, ps, Ident, bias=pb_sb[:, ec : ec + 1])
            else:
                nc.any.tensor_add(dst, dst, ps)
        if outT is not None:
            nc.sync.dma_start(
                out=outT[b, 128 * ec : 128 * (ec + 1)], in_=out_acc[:, ec]
            )


def _host_prep(x, qkv_w, proj_w, proj_b):
    import ml_dtypes

    xT = np.ascontiguousarray(x.transpose(0, 2, 1))              # [B, DIM, N]
    wqkvT = np.ascontiguousarray(qkv_w.transpose(0, 2, 1))       # [G, 256, 768]
    pwT = np.ascontiguousarray(proj_w.T).astype(ml_dtypes.bfloat16)  # [DIM, DIM]
    pb = np.ascontiguousarray(proj_b.reshape(DIM // P, P).T)     # [128, 8]
    return xT, wqkvT, pwT, pb


def kernel(x, qkv_w, proj_w, proj_b):
    from concourse.bass_utils import run_bass_kernel_spmd

    x = np.asarray(x, dtype=np.float32)
    qkv_w = np.asarray(qkv_w, dtype=np.float32)
    proj_w = np.asarray(proj_w, dtype=np.float32)
    proj_b = np.asarray(proj_b, dtype=np.float32)

    xT, wqkvT, pwT, pb = _host_prep(x, qkv_w, proj_w, proj_b)

    nc = build_nc()
    in_maps = [
        {
            "xT": np.ascontiguousarray(xT[c * B_PER : (c + 1) * B_PER]),
            "wqkvT": wqkvT,
            "pwT": pwT,
            "pb": pb,
        }
        for c in range(N_CORES)
    ]
    res = run_bass_kernel_spmd(nc, in_maps, core_ids=list(range(N_CORES)))
    shards = [res.results[c]["outT"] for c in range(N_CORES)]
    outT = np.concatenate(shards, axis=0)          # [B, DIM, N] bf16
    return np.ascontiguousarray(
        outT.astype(np.float32).transpose(0, 2, 1)
    )


if __name__ == "__main__":
    import sys

    if len(sys.argv) > 1 and sys.argv[1] == "build":
        nc = build_nc()
        print("build OK")
